# revision 1
# baseline (speedup 1.0000x reference)
"""Global-average-pool + sigmoid channel scores on 8 trn2 NeuronCores.

Problem: x (32, 64, 224, 224) f32 -> sigmoid(mean(x, axes=(0,2,3))) broadcast
to (32, 64).  Data-parallel over batch: core i reduces the contiguous shard
x[4i:4i+4], cores AllReduce the 128-length per-partition partial sums, and each
core finishes sigmoid + broadcast locally (output replicated; host takes
core 0's copy).
"""

import numpy as np

try:
    import concourse.bass as bass  # noqa: F401
except ImportError:  # pragma: no cover - fallback when site path is absent
    import sys

    for p in ("/opt/trn_rl_repo", "/root/.axon_site/_ro/trn_rl_repo"):
        if p not in sys.path:
            sys.path.insert(0, p)

import concourse.bass as bass
import concourse.bacc as bacc
import concourse.mybir as mybir
import concourse.tile as tile
from concourse.bass_utils import run_bass_kernel_spmd

N_CORES = 8
B, C, H, W = 32, 64, 224, 224
B_LOC = B // N_CORES            # 4 batches per core
ROWS = B_LOC * C                # 256 (b_loc, c) rows per core
HW = H * W                      # 50176 spatial elements per row
N_PTILES = ROWS // 128          # 2 partition tiles of 128 rows
CHUNK = 6272                    # 50176 = 8 * 6272; 3.2 MB per DMA tile
N_CHUNKS = HW // CHUNK          # 8 free-dim chunks per partition tile
N_RED = N_PTILES * N_CHUNKS     # 16 reduce slots
MEAN_SCALE = 1.0 / (B * HW)     # mean over batch+spatial = 32*50176 elems

_CACHE = {}


def _build():
    nc = bacc.Bacc(
        "TRN2",
        target_bir_lowering=False,
        debug=False,
        num_devices=N_CORES,
    )
    xs = nc.dram_tensor("xs", [ROWS, HW], mybir.dt.float32, kind="ExternalInput")
    out = nc.dram_tensor("out", [B, C], mybir.dt.float32, kind="ExternalOutput")
    xs_ap = xs.ap()
    out_ap = out.ap()

    with tile.TileContext(nc) as tc:
        with (
            tc.tile_pool(name="data", bufs=4) as data_pool,
            tc.tile_pool(name="small", bufs=1) as small_pool,
            tc.tile_pool(name="dram", bufs=1, space="DRAM") as dram_pool,
        ):
            stats = small_pool.tile([128, N_RED], mybir.dt.float32)
            for n in range(N_PTILES):
                for j in range(N_CHUNKS):
                    t_in = data_pool.tile([128, CHUNK], mybir.dt.float32, tag="data")
                    nc.sync.dma_start(
                        out=t_in[:, :],
                        in_=xs_ap[n * 128 : (n + 1) * 128, j * CHUNK : (j + 1) * CHUNK],
                    )
                    i = n * N_CHUNKS + j
                    nc.vector.reduce_sum(
                        out=stats[:, i : i + 1],
                        in_=t_in[:, :],
                        axis=mybir.AxisListType.X,
                    )

            psum = small_pool.tile([128, 1], mybir.dt.float32)
            nc.vector.reduce_sum(
                out=psum[:, :], in_=stats[:, :], axis=mybir.AxisListType.X
            )

            cc_in = dram_pool.tile([128, 1], mybir.dt.float32)
            cc_out = dram_pool.tile([128, 1], mybir.dt.float32)
            nc.sync.dma_start(out=cc_in[:, :], in_=psum[:, :])
            nc.gpsimd.collective_compute(
                "AllReduce",
                mybir.AluOpType.add,
                replica_groups=[list(range(N_CORES))],
                ins=[cc_in[:, :].opt()],
                outs=[cc_out[:, :].opt()],
            )

            # 128 reduced sums live contiguously in DRAM; reload on 1 partition.
            row = small_pool.tile([1, 128], mybir.dt.float32)
            nc.sync.dma_start(out=row[:, :], in_=cc_out[:, :].rearrange("p o -> o p"))

            # row[p] covers batches {p//64, p//64+2, ...}; fold the two halves
            # so col c = full sum over batch+spatial for channel c.
            folded = small_pool.tile([1, C], mybir.dt.float32)
            nc.vector.tensor_add(folded[:, :], row[:, 0:C], row[:, C : 2 * C])

            scores = small_pool.tile([1, C], mybir.dt.float32)
            nc.scalar.activation(
                scores[:, :],
                folded[:, :],
                mybir.ActivationFunctionType.Sigmoid,
                scale=MEAN_SCALE,
            )

            rep = small_pool.tile([B, C], mybir.dt.float32)
            nc.gpsimd.partition_broadcast(rep[:, :], scores[:, :])
            nc.sync.dma_start(out=out_ap[:, :], in_=rep[:, :])

    nc.compile()
    return nc


def _get_nc():
    if "nc" not in _CACHE:
        _CACHE["nc"] = _build()
    return _CACHE["nc"]


def _in_maps(x: np.ndarray):
    x = np.ascontiguousarray(np.asarray(x, dtype=np.float32))
    return [
        {"xs": x[i * B_LOC : (i + 1) * B_LOC].reshape(ROWS, HW)}
        for i in range(N_CORES)
    ]


def _run(x: np.ndarray, **kwargs):
    return run_bass_kernel_spmd(_get_nc(), _in_maps(x), list(range(N_CORES)), **kwargs)


def kernel(x: np.ndarray) -> np.ndarray:
    res = _run(x)
    return np.asarray(res.results[0]["out"], dtype=np.float32)


# revision 4
# speedup vs baseline: 1.0380x; 1.0380x over previous
"""Global-average-pool + sigmoid channel scores on 8 trn2 NeuronCores.

Problem: x (32, 64, 224, 224) f32 -> sigmoid(mean(x, axes=(0,2,3))) broadcast
to (32, 64).  Data-parallel over batch: core i reduces the contiguous shard
x[4i:4i+4], cores AllReduce the 128-length per-partition partial sums, and each
core finishes sigmoid + broadcast locally (output replicated; host takes
core 0's copy).
"""

import numpy as np

try:
    import concourse.bass as bass  # noqa: F401
except ImportError:  # pragma: no cover - fallback when site path is absent
    import sys

    for p in ("/opt/trn_rl_repo", "/root/.axon_site/_ro/trn_rl_repo"):
        if p not in sys.path:
            sys.path.insert(0, p)

import concourse.bass as bass
import concourse.bacc as bacc
import concourse.mybir as mybir
import concourse.tile as tile
from concourse.bass_utils import run_bass_kernel_spmd

N_CORES = 8
B, C, H, W = 32, 64, 224, 224
B_LOC = B // N_CORES            # 4 batches per core
ROWS = B_LOC * C                # 256 (b_loc, c) rows per core
HW = H * W                      # 50176 spatial elements per row
N_PTILES = ROWS // 128          # 2 partition tiles of 128 rows
CHUNK = 6272                    # 50176 = 8 * 6272; 3.2 MB per DMA tile
N_CHUNKS = HW // CHUNK          # 8 free-dim chunks per partition tile
N_RED = N_PTILES * N_CHUNKS     # 16 reduce slots
MEAN_SCALE = 1.0 / (B * HW)     # mean over batch+spatial = 32*50176 elems

_CACHE = {}


def _build():
    nc = bacc.Bacc(
        "TRN2",
        target_bir_lowering=False,
        debug=False,
        num_devices=N_CORES,
    )
    xs = nc.dram_tensor("xs", [ROWS, HW], mybir.dt.float32, kind="ExternalInput")
    out = nc.dram_tensor("out", [B, C], mybir.dt.float32, kind="ExternalOutput")
    xs_ap = xs.ap()
    out_ap = out.ap()

    with tile.TileContext(nc) as tc:
        with (
            tc.tile_pool(name="data", bufs=4) as data_pool,
            tc.tile_pool(name="small", bufs=1) as small_pool,
            tc.tile_pool(name="dram", bufs=1, space="DRAM") as dram_pool,
        ):
            stats = small_pool.tile([128, N_RED], mybir.dt.float32)
            for n in range(N_PTILES):
                for j in range(N_CHUNKS):
                    t_in = data_pool.tile([128, CHUNK], mybir.dt.float32, tag="data")
                    nc.sync.dma_start(
                        out=t_in[:, :],
                        in_=xs_ap[n * 128 : (n + 1) * 128, j * CHUNK : (j + 1) * CHUNK],
                    )
                    i = n * N_CHUNKS + j
                    nc.vector.reduce_sum(
                        out=stats[:, i : i + 1],
                        in_=t_in[:, :],
                        axis=mybir.AxisListType.X,
                    )

            # Warm-up collective with no data deps: it runs while the main
            # loop streams, absorbing the cross-core alignment + ncfw
            # first-call latency so the real collective below starts hot.
            warm_in = dram_pool.tile([1, 1], mybir.dt.float32)
            warm_out = dram_pool.tile([N_CORES, 1], mybir.dt.float32)
            wz = small_pool.tile([1, 1], mybir.dt.float32)
            nc.vector.memset(wz[:, :], 0.0)
            nc.sync.dma_start(out=warm_in[:, :], in_=wz[:, :])
            nc.gpsimd.collective_compute(
                "AllGather",
                mybir.AluOpType.bypass,
                replica_groups=[list(range(N_CORES))],
                ins=[warm_in[:, :].opt()],
                outs=[warm_out[:, :].opt()],
            )

            psum = small_pool.tile([128, 1], mybir.dt.float32)
            nc.vector.reduce_sum(
                out=psum[:, :], in_=stats[:, :], axis=mybir.AxisListType.X
            )

            cc_in = dram_pool.tile([128, 1], mybir.dt.float32)
            cc_out = dram_pool.tile([1, N_CORES * 128], mybir.dt.float32)
            nc.sync.dma_start(out=cc_in[:, :], in_=psum[:, :])
            nc.gpsimd.collective_compute(
                "AllGather",
                mybir.AluOpType.bypass,
                replica_groups=[list(range(N_CORES))],
                ins=[cc_in[:, :].opt()],
                outs=[cc_out[:, :].opt()],
            )

            # All 8 ranks' 128 partial sums live contiguously in DRAM
            # (rank-major); reload on 1 partition and halve 4 times:
            # 1024 -> 512 -> 256 -> 128 folds ranks, 128 -> 64 folds the
            # two batch halves, leaving per-channel totals.
            row = small_pool.tile([1, N_CORES * 128], mybir.dt.float32)
            nc.sync.dma_start(out=row[:, :], in_=cc_out[:, :])
            folded = small_pool.tile([1, C], mybir.dt.float32)
            cur = row
            width = N_CORES * 128
            while width > C:
                width //= 2
                dst = folded if width == C else small_pool.tile(
                    [1, width], mybir.dt.float32, tag=f"fold{width}"
                )
                nc.vector.tensor_add(
                    dst[:, :], cur[:, 0:width], cur[:, width : 2 * width]
                )
                cur = dst

            scores = small_pool.tile([1, C], mybir.dt.float32)
            nc.scalar.activation(
                scores[:, :],
                folded[:, :],
                mybir.ActivationFunctionType.Sigmoid,
                scale=MEAN_SCALE,
            )

            rep = small_pool.tile([B, C], mybir.dt.float32)
            nc.gpsimd.partition_broadcast(rep[:, :], scores[:, :])
            nc.sync.dma_start(out=out_ap[:, :], in_=rep[:, :])

    nc.compile()
    return nc


def _get_nc():
    if "nc" not in _CACHE:
        _CACHE["nc"] = _build()
    return _CACHE["nc"]


def _in_maps(x: np.ndarray):
    x = np.ascontiguousarray(np.asarray(x, dtype=np.float32))
    return [
        {"xs": x[i * B_LOC : (i + 1) * B_LOC].reshape(ROWS, HW)}
        for i in range(N_CORES)
    ]


def _run(x: np.ndarray, **kwargs):
    return run_bass_kernel_spmd(_get_nc(), _in_maps(x), list(range(N_CORES)), **kwargs)


def kernel(x: np.ndarray) -> np.ndarray:
    res = _run(x)
    return np.asarray(res.results[0]["out"], dtype=np.float32)


# revision 5
# speedup vs baseline: 1.1197x; 1.0786x over previous
"""Global-average-pool + sigmoid channel scores on 8 trn2 NeuronCores.

Problem: x (32, 64, 224, 224) f32 -> sigmoid(mean(x, axes=(0,2,3))) broadcast
to (32, 64).  Data-parallel over batch: core i reduces the contiguous shard
x[4i:4i+4], cores AllReduce the 128-length per-partition partial sums, and each
core finishes sigmoid + broadcast locally (output replicated; host takes
core 0's copy).
"""

import numpy as np

try:
    import concourse.bass as bass  # noqa: F401
except ImportError:  # pragma: no cover - fallback when site path is absent
    import sys

    for p in ("/opt/trn_rl_repo", "/root/.axon_site/_ro/trn_rl_repo"):
        if p not in sys.path:
            sys.path.insert(0, p)

import concourse.bass as bass
import concourse.bacc as bacc
import concourse.mybir as mybir
import concourse.tile as tile
from concourse.bass_utils import run_bass_kernel_spmd

N_CORES = 8
B, C, H, W = 32, 64, 224, 224
B_LOC = B // N_CORES            # 4 batches per core
ROWS = B_LOC * C                # 256 (b_loc, c) rows per core
HW = H * W                      # 50176 spatial elements per row
N_PTILES = ROWS // 128          # 2 partition tiles of 128 rows
CHUNK = 6272                    # 50176 = 8 * 6272; 3.2 MB per DMA tile
N_CHUNKS = HW // CHUNK          # 8 free-dim chunks per partition tile
N_RED = N_PTILES * N_CHUNKS     # 16 reduce slots
MEAN_SCALE = 1.0 / (B * HW)     # mean over batch+spatial = 32*50176 elems

_CACHE = {}


def _build():
    nc = bacc.Bacc(
        "TRN2",
        target_bir_lowering=False,
        debug=False,
        num_devices=N_CORES,
    )
    xs = nc.dram_tensor("xs", [ROWS, HW], mybir.dt.float32, kind="ExternalInput")
    out = nc.dram_tensor("out", [B, C], mybir.dt.float32, kind="ExternalOutput")
    xs_ap = xs.ap()
    out_ap = out.ap()

    with tile.TileContext(nc) as tc:
        with (
            tc.tile_pool(name="data", bufs=4) as data_pool,
            tc.tile_pool(name="small", bufs=1) as small_pool,
            tc.tile_pool(name="dram", bufs=1, space="DRAM") as dram_pool,
        ):
            # Warm-up collective with no data deps, entirely on gpsimd (idle
            # engine) so it fires immediately: it runs while the main loop
            # streams, absorbing the cross-core alignment + ncfw first-call
            # latency so the real collective below starts hot.
            warm_in = dram_pool.tile([1, 1], mybir.dt.float32)
            warm_out = dram_pool.tile([N_CORES, 1], mybir.dt.float32)
            wz = small_pool.tile([1, 1], mybir.dt.float32)
            nc.gpsimd.memset(wz[:, :], 0.0)
            nc.gpsimd.dma_start(out=warm_in[:, :], in_=wz[:, :])
            nc.gpsimd.collective_compute(
                "AllGather",
                mybir.AluOpType.bypass,
                replica_groups=[list(range(N_CORES))],
                ins=[warm_in[:, :].opt()],
                outs=[warm_out[:, :].opt()],
            )

            # Streaming phase: 15 full chunks + a split final chunk (so the
            # last reduce drains quickly after the last DMA).  DMAs alternate
            # between the two HWDGE rings (sync / scalar issuers).
            TAIL_SPLIT = 4
            TAIL = CHUNK // TAIL_SPLIT
            pieces = []  # (row_tile_idx, col_start, width)
            for n in range(N_PTILES):
                for j in range(N_CHUNKS):
                    if n == N_PTILES - 1 and j == N_CHUNKS - 1:
                        for k in range(TAIL_SPLIT):
                            pieces.append((n, j * CHUNK + k * TAIL, TAIL))
                    else:
                        pieces.append((n, j * CHUNK, CHUNK))

            stats = small_pool.tile([128, len(pieces)], mybir.dt.float32)
            dma_engines = (nc.sync, nc.scalar)
            for i, (n, col, width) in enumerate(pieces):
                t_in = data_pool.tile([128, width], mybir.dt.float32, tag="data")
                dma_engines[i % 2].dma_start(
                    out=t_in[:, 0:width],
                    in_=xs_ap[n * 128 : (n + 1) * 128, col : col + width],
                )
                nc.vector.reduce_sum(
                    out=stats[:, i : i + 1],
                    in_=t_in[:, 0:width],
                    axis=mybir.AxisListType.X,
                )

            psum = small_pool.tile([128, 1], mybir.dt.float32)
            nc.vector.reduce_sum(
                out=psum[:, :], in_=stats[:, :], axis=mybir.AxisListType.X
            )

            cc_in = dram_pool.tile([128, 1], mybir.dt.float32)
            cc_out = dram_pool.tile([1, N_CORES * 128], mybir.dt.float32)
            nc.sync.dma_start(out=cc_in[:, :], in_=psum[:, :])
            nc.gpsimd.collective_compute(
                "AllGather",
                mybir.AluOpType.bypass,
                replica_groups=[list(range(N_CORES))],
                ins=[cc_in[:, :].opt()],
                outs=[cc_out[:, :].opt()],
            )

            # All 8 ranks' 128 partial sums live contiguously in DRAM
            # (rank-major); reload on 1 partition and halve 4 times:
            # 1024 -> 512 -> 256 -> 128 folds ranks, 128 -> 64 folds the
            # two batch halves, leaving per-channel totals.
            row = small_pool.tile([1, N_CORES * 128], mybir.dt.float32)
            nc.sync.dma_start(out=row[:, :], in_=cc_out[:, :])
            folded = small_pool.tile([1, C], mybir.dt.float32)
            cur = row
            width = N_CORES * 128
            while width > C:
                width //= 2
                dst = folded if width == C else small_pool.tile(
                    [1, width], mybir.dt.float32, tag=f"fold{width}"
                )
                nc.vector.tensor_add(
                    dst[:, :], cur[:, 0:width], cur[:, width : 2 * width]
                )
                cur = dst

            scores = small_pool.tile([1, C], mybir.dt.float32)
            nc.scalar.activation(
                scores[:, :],
                folded[:, :],
                mybir.ActivationFunctionType.Sigmoid,
                scale=MEAN_SCALE,
            )

            rep = small_pool.tile([B, C], mybir.dt.float32)
            nc.gpsimd.partition_broadcast(rep[:, :], scores[:, :])
            nc.sync.dma_start(out=out_ap[:, :], in_=rep[:, :])

    nc.compile()
    return nc


def _get_nc():
    if "nc" not in _CACHE:
        _CACHE["nc"] = _build()
    return _CACHE["nc"]


def _in_maps(x: np.ndarray):
    x = np.ascontiguousarray(np.asarray(x, dtype=np.float32))
    return [
        {"xs": x[i * B_LOC : (i + 1) * B_LOC].reshape(ROWS, HW)}
        for i in range(N_CORES)
    ]


def _run(x: np.ndarray, **kwargs):
    return run_bass_kernel_spmd(_get_nc(), _in_maps(x), list(range(N_CORES)), **kwargs)


def kernel(x: np.ndarray) -> np.ndarray:
    res = _run(x)
    return np.asarray(res.results[0]["out"], dtype=np.float32)
